# revision 27
# baseline (speedup 1.0000x reference)
"""Causal self-attention (GQA + QK-RMSNorm + RoPE + q_gain) on 8 Trainium2 cores.

Sharding: 8 cores = 2 (batch) x 4 (KV head group).  Core c handles batch
c//4 and KV head g=c%4, i.e. Q heads 4g..4g+3.  Each core computes its
heads' attention and a partial output projection (its 512 columns of the
attention output against the matching 512 rows of Wproj^T); the host sums
the 4 partials per batch.

Implementation notes (fp8/bf16 mixed precision):
- QKV projections run as 3-term fp8e4m3 DoubleRow matmuls: x ~ x8 + xr8
  (quantized host-side at scale 2^4), W ~ w8 + wr8 (scale 2^9), and the
  x8@w8 + x8@wr8 + xr8@w8 terms accumulate in PSUM (xr8@wr8 ~ 0.06% is
  dropped).  DoubleRow packs two 128-deep contraction tiles per
  instruction at half cost.  The 2^13 product scale cancels inside
  QK-RMSNorm; for V it is folded into Wproj on the host.
- V is produced directly in [token, hd] layout (DoubleRow, N=128), so no
  PE transposes are needed.
- Attention scores run in f32r; exp writes bf16 pt tiles; the softmax
  denominator accumulates pt tiles on the DVE (bf16, 2x rate) and needs
  only ONE ones-matmul per (head, query-slice) instead of one per key
  tile.  PV and the output projection run in bf16 (full PE rate).
- Causal diagonal blocks are padded to a moving size >= 256 (f32r matmuls
  below 256 run at 1/4 rate).
- 1/sqrt(v) in RMSNorm is computed as exp(-0.5*ln(v)) so every activation
  used (square/ln/exp/copy) lives in ONE act-function table set -- no
  table reloads on the activation engine.
- Software-pipelined emission: attention segments are activation-bound
  (exp stream), so PE-dense work is interleaved into them as filler:
      QKV0; QKV1; attn0*QKV2; attn1*QKV3; attn2*proj0;
      attn3*(proj1+proj2); proj3
  The per-engine in-order queues then keep the tensor engine busy while
  exps stream.

All shapes hardcoded for B=2, S=2048, D=2048, H=16, KVH=4, HD=128.
"""

import numpy as np

B, S, D = 2, 2048, 2048
H, KVH = 16, 4
HD = 128  # head dim
G = H // KVH  # q heads per kv group = 4
NCORES = 8
ROPE_BASE = 10000.0
EPS = 1e-6

P = 128          # partitions
SL = 512         # token slice
NSL = S // SL    # 4
DK = D // P      # 16 contraction subtiles
NPAIR = DK // 2  # 8 DoubleRow pairs

SX = 2.0 ** 4    # x quantization scale
SW = 2.0 ** 9    # weight quantization scale
SXW = SX * SW    # product scale 2^13

_CACHE = {}


def _build_program():
    """Build + compile the (single, SPMD) Bass program. Returns nc."""
    from contextlib import ExitStack

    import concourse.bass as bass
    import concourse.tile as tile
    from concourse import bacc, mybir

    f32 = mybir.dt.float32
    f32r = mybir.dt.float32r
    f8e4 = mybir.dt.float8e4
    bf16 = mybir.dt.bfloat16
    AF = mybir.ActivationFunctionType
    OP = mybir.AluOpType
    DR = mybir.MatmulPerfMode.DoubleRow

    nc = bacc.Bacc("TRN2", target_bir_lowering=False)

    from concourse.hw_specs import get_activation_tables
    act_set_id = list(get_activation_tables(nc.m.arch)).index(
        "natural_log_exp_and_others")

    x8_d = nc.dram_tensor("x8", [D, S], f8e4, kind="ExternalInput").ap()
    xr8_d = nc.dram_tensor("xr8", [D, S], f8e4, kind="ExternalInput").ap()
    wq8_d = nc.dram_tensor("wq8", [D, G * HD], f8e4, kind="ExternalInput").ap()
    wqr8_d = nc.dram_tensor("wqr8", [D, G * HD], f8e4, kind="ExternalInput").ap()
    wk8_d = nc.dram_tensor("wk8", [D, HD], f8e4, kind="ExternalInput").ap()
    wkr8_d = nc.dram_tensor("wkr8", [D, HD], f8e4, kind="ExternalInput").ap()
    wv8_d = nc.dram_tensor("wv8", [D, HD], f8e4, kind="ExternalInput").ap()
    wvr8_d = nc.dram_tensor("wvr8", [D, HD], f8e4, kind="ExternalInput").ap()
    wpT_d = nc.dram_tensor("wpT", [G * HD, D], bf16, kind="ExternalInput").ap()
    cosT_d = nc.dram_tensor("cosT", [HD, S], f32, kind="ExternalInput").ap()
    sinT_d = nc.dram_tensor("sinT", [HD, S], f32, kind="ExternalInput").ap()
    jT_d = nc.dram_tensor("jT", [HD, HD], f32r, kind="ExternalInput").ap()
    qgain_d = nc.dram_tensor("qgain", [1, G], f32, kind="ExternalInput").ap()
    y_d = nc.dram_tensor("y", [S, D], f32, kind="ExternalOutput").ap()

    x8_3 = x8_d.rearrange("(o p) s -> p o s", p=P)
    xr8_3 = xr8_d.rearrange("(o p) s -> p o s", p=P)
    wq8_3 = wq8_d.rearrange("(o p) m -> p o m", p=P)
    wqr8_3 = wqr8_d.rearrange("(o p) m -> p o m", p=P)

    with tile.TileContext(nc) as tc, ExitStack() as top:
        # Pre-load the one act-function table covering every activation we
        # use (square/ln/exp/copy); the compile-time fixpoint pass then
        # inserts no further table reloads.
        nc.scalar.add_instruction(mybir.InstLoadActFuncSet(
            name=nc.get_next_instruction_name(),
            act_func_set_id=act_set_id, ins=[], outs=[]))

        res = top.enter_context(tc.tile_pool(name="resident", bufs=1))
        xtp = top.enter_context(tc.tile_pool(name="xt", bufs=2))
        tmp = top.enter_context(tc.tile_pool(name="tmp", bufs=2))
        ptp = top.enter_context(tc.tile_pool(name="pt", bufs=6))
        ptsp = top.enter_context(tc.tile_pool(name="ptsum", bufs=3))
        rbp = top.enter_context(tc.tile_pool(name="rb", bufs=2))
        ysp = top.enter_context(tc.tile_pool(name="ysb", bufs=6))

        ps_big = top.enter_context(tc.tile_pool(name="psbig", bufs=2, space="PSUM"))
        ps_sc = top.enter_context(tc.tile_pool(name="pssc", bufs=2, space="PSUM"))
        ps_o = top.enter_context(tc.tile_pool(name="pso", bufs=2, space="PSUM"))
        ps_nm = top.enter_context(tc.tile_pool(name="psnm", bufs=1, space="PSUM"))
        ps_vo = top.enter_context(tc.tile_pool(name="psvo", bufs=1, space="PSUM"))

        # ---- startup DMAs: K weights (+residuals) first so the tensor
        # engine can start, then x slice 0 interleaved with xr.  DMA count
        # is kept low: HWDGE costs ~625ns per transfer.  cos/sin stream in
        # per-slice chunks alongside x; wp is deferred (first use is the
        # jq=0 projection, interleaved into the attn(2) segment). ----
        XC = 4  # dk granularity of x-chunk DMAs
        cos_sb = res.tile([HD, S], f32)
        sin_sb = res.tile([HD, S], f32)

        def load_cs(js):
            ssl = slice(js * SL, (js + 1) * SL)
            nc.sync.dma_start(cos_sb[:, ssl], cosT_d[:, ssl])
            nc.sync.dma_start(sin_sb[:, ssl], sinT_d[:, ssl])

        wk8 = res.tile([P, DK, HD], f8e4)
        nc.sync.dma_start(wk8[:, 0:2, :],
                          wk8_d.rearrange("(o p) m -> p o m", p=P)[:, 0:2, :])
        nc.sync.dma_start(wk8[:, 2:, :],
                          wk8_d.rearrange("(o p) m -> p o m", p=P)[:, 2:, :])
        wkr8 = res.tile([P, DK, HD], f8e4)
        nc.sync.dma_start(wkr8[:], wkr8_d.rearrange("(o p) m -> p o m", p=P))
        jT = res.tile([HD, HD], f32r)
        nc.sync.dma_start(jT[:], jT_d[:])
        qgain = res.tile([P, G], f32)
        nc.gpsimd.dma_start(qgain[:], qgain_d.to_broadcast([P, G]))
        load_cs(0)
        xts0 = xtp.tile([P, DK, SL], f8e4, tag="x8", name="x8_0")
        xrs0 = xtp.tile([P, DK, SL], f8e4, tag="xr8", name="xr8_0")
        for c in range(0, DK, XC):
            nc.sync.dma_start(xts0[:, c:c + XC, :], x8_3[:, c:c + XC, 0:SL])
            nc.sync.dma_start(xrs0[:, c:c + XC, :], xr8_3[:, c:c + XC, 0:SL])
        wv8 = res.tile([P, DK, HD], f8e4)
        nc.sync.dma_start(wv8[:], wv8_d.rearrange("(o p) m -> p o m", p=P))
        wvr8 = res.tile([P, DK, HD], f8e4)
        nc.sync.dma_start(wvr8[:], wvr8_d.rearrange("(o p) m -> p o m", p=P))
        wq8 = res.tile([P, DK, G * HD], f8e4)
        wqr8 = res.tile([P, DK, G * HD], f8e4)
        for c in range(0, DK, DK // 2):
            cs = slice(c, c + DK // 2)
            nc.sync.dma_start(wq8[:, cs, :], wq8_3[:, cs, :])
            nc.sync.dma_start(wqr8[:, cs, :], wqr8_3[:, cs, :])
        wp_sb = res.tile([P, G, D], bf16)

        def load_wp():
            nc.sync.dma_start(wp_sb[:], wpT_d.rearrange("(o p) m -> p o m", p=P))

        # ---- small constants ----
        ones_f = res.tile([P, P], f32)
        nc.vector.memset(ones_f[:], 1.0)
        ones_r = res.tile([P, P], f32r)
        nc.vector.tensor_copy(ones_r[:], ones_f[:])
        ones_b = res.tile([P, P], bf16)
        nc.vector.tensor_copy(ones_b[:], ones_f[:])
        eps_t = res.tile([P, 1], f32)
        nc.vector.memset(eps_t[:], EPS * SXW * SXW)  # eps * 2^26

        # ---- resident tensors ----
        qT = [res.tile([P, S], f32r, tag=f"qT{h}", name=f"qT{h}") for h in range(G)]
        kT = res.tile([P, S], f32r)
        v_sb = res.tile([P, S // P, HD], bf16)  # natural [s_inner, s_tile, hd]
        oT = [res.tile([P, S], bf16, tag=f"oT{h}", name=f"oT{h}") for h in range(G)]

        xt_sb = {0: (xts0, xrs0)}

        def load_x(js):
            ssl = slice(js * SL, (js + 1) * SL)
            x8t = xtp.tile([P, DK, SL], f8e4, tag="x8", name=f"x8_{js}")
            xr8t = xtp.tile([P, DK, SL], f8e4, tag="xr8", name=f"xr8_{js}")
            for c in range(0, DK, XC):
                nc.sync.dma_start(x8t[:, c:c + XC, :], x8_3[:, c:c + XC, ssl])
                nc.sync.dma_start(xr8t[:, c:c + XC, :], xr8_3[:, c:c + XC, ssl])
            load_cs(js)
            xt_sb[js] = (x8t, xr8t)

        def qkv_group(out_ps, w8t, wr8t, js, mlo, mhi):
            """3-term fp8 DoubleRow accumulation: (x8+xr8)@(w8+wr8) minus
            the xr8@wr8 term.  Pair-major so each arriving x chunk pair
            feeds all three terms."""
            x8t, xr8t = xt_sb[js]
            for pr in range(NPAIR):
                for ti, (wt, xt) in enumerate(
                        [(w8t, x8t), (wr8t, x8t), (w8t, xr8t)]):
                    nc.tensor.matmul(
                        out_ps[:],
                        wt[:, 2 * pr:2 * pr + 2, mlo:mhi],
                        xt[:, 2 * pr:2 * pr + 2, 0:SL],
                        start=(pr, ti) == (0, 0), stop=(pr, ti) == (NPAIR - 1, 2),
                        perf_mode=DR)

        def norm_rope(src_ps, dst, js, gain_ap):
            """RMS-normalize (+optional gain) and RoPE a [128, SL] head block.

            src_ps holds the raw fp8-path projection at scale 2^13; the
            norm factor is computed at the same scale so the output comes
            out at true scale (eps folded in at 2^26).  1/sqrt via
            exp(-0.5*ln(v)) keeps all activations in one table set."""
            sq = tmp.tile([P, SL], f32r, tag="sq")
            nc.scalar.square(sq[:], src_ps[:])
            ssq = ps_nm.tile([P, SL], f32, tag="nm", name="ssq")
            nc.tensor.matmul(ssq[:], ones_r[:], sq[:], start=True, stop=True)
            lnv = tmp.tile([P, SL], f32, tag="lnv")
            nc.scalar.activation(lnv[:], ssq[:], AF.Ln,
                                 bias=eps_t[:], scale=1.0 / HD)
            fb = tmp.tile([P, SL], f32, tag="fb")
            nc.scalar.activation(fb[:], lnv[:], AF.Exp, scale=-0.5)
            if gain_ap is not None:
                nc.vector.tensor_scalar_mul(fb[:], fb[:], gain_ap)
            qn = tmp.tile([P, SL], f32r, tag="qn")
            nc.vector.tensor_mul(qn[:], src_ps[:], fb[:])
            qj = ps_nm.tile([P, SL], f32, tag="nm", name="qj")
            nc.tensor.matmul(qj[:], jT[:], qn[:], start=True, stop=True)
            c = cos_sb[:, js * SL:(js + 1) * SL]
            s = sin_sb[:, js * SL:(js + 1) * SL]
            t1 = tmp.tile([P, SL], f32, tag="t1")
            t2 = tmp.tile([P, SL], f32, tag="t2")
            nc.gpsimd.tensor_mul(t1[:], qn[:], c)
            nc.vector.tensor_mul(t2[:], qj[:], s)
            nc.vector.tensor_add(dst, t1[:], t2[:])

        def emit_k(js):
            k_ps = ps_big.tile([P, SL], f32, tag="big", name=f"k_ps{js}")
            qkv_group(k_ps, wk8, wkr8, js, 0, HD)
            norm_rope(k_ps, kT[:, js * SL:(js + 1) * SL], js, None)

        def emit_v(js):
            x8t, xr8t = xt_sb[js]
            v_ps = ps_vo.tile([P, SL // P, HD], f32, tag="vo")
            for t in range(SL // P):
                terms = [(wv8, x8t), (wvr8, x8t), (wv8, xr8t)]
                for ti, (wt, xt) in enumerate(terms):
                    for pr in range(NPAIR):
                        nc.tensor.matmul(
                            v_ps[:, t, :],
                            xt[:, 2 * pr:2 * pr + 2, t * P:(t + 1) * P],
                            wt[:, 2 * pr:2 * pr + 2, :],
                            start=(ti, pr) == (0, 0),
                            stop=(ti, pr) == (2, NPAIR - 1),
                            perf_mode=DR)
            nc.scalar.copy(
                v_sb[:, js * (SL // P):(js + 1) * (SL // P), :], v_ps[:])

        def emit_q(js, h):
            q_ps = ps_big.tile([P, SL], f32, tag="big", name=f"q_ps{js}_{h}")
            qkv_group(q_ps, wq8, wqr8, js, h * HD, (h + 1) * HD)
            norm_rope(q_ps, qT[h][:, js * SL:(js + 1) * SL], js,
                      qgain[:, h:h + 1])

        def qkv_units(js, with_load):
            units = []
            if with_load:
                units.append(lambda: load_x(js))
            units.append(lambda: emit_k(js))
            units.append(lambda: emit_v(js))
            for h in range(G):
                units.append(lambda h=h: emit_q(js, h))
            return units

        def proj_chunk(jq, st, os_, width=SL):
            st_g = jq * (SL // P) + st
            for w0 in range(0, SL, width):
                y_ps = ps_big.tile([P, width], f32, tag="big",
                                   name=f"y_{st_g}_{os_}_{w0}")
                for h in range(G):
                    nc.tensor.matmul(
                        y_ps[:], oT[h][:, st_g * P:(st_g + 1) * P],
                        wp_sb[:, h, os_ * SL + w0:os_ * SL + w0 + width],
                        start=(h == 0), stop=(h == G - 1))
                y_sb = ysp.tile([P, width], f32, tag="ysb")
                if (os_ + w0) % 2 == 0:
                    nc.scalar.copy(y_sb[:], y_ps[:])
                else:
                    nc.vector.tensor_copy(y_sb[:], y_ps[:])
                nc.sync.dma_start(
                    y_d[st_g * P:(st_g + 1) * P,
                        os_ * SL + w0:os_ * SL + w0 + width], y_sb[:])

        def proj_units(jq):
            return [lambda st=st, os_=os_: proj_chunk(jq, st, os_)
                    for st in range(SL // P) for os_ in range(D // SL)]

        def attn(jq, filler):
            """Attention for query slice jq (all 4 heads), interleaving
            `filler` emission units as PE-dense work behind the exp
            stream.  The score matmul runs one step ahead of the
            exp/PV consumption so PV never heads the PE queue while its
            exp is still in flight."""
            nsteps = G * (4 * jq + 4)
            pace = max(1, -(-nsteps // max(len(filler), 1)))
            nfill = len(filler)
            step = 0
            fi = 0
            ilast = 4 * jq + 3

            def spans(i):
                delta = i - 4 * jq
                lo_c = P * delta if 0 <= delta <= 3 else 0
                return delta, lo_c, min(lo_c, SL - 256)

            def emit_sc(h, i):
                # f32r score matmul padded to >= 256 moving columns
                # (below 256 f32r runs at 1/4 rate; pad is never read)
                _, _, sc_lo = spans(i)
                sc = ps_sc.tile([P, SL], f32, tag="sc")
                nc.tensor.matmul(sc[:, sc_lo:SL], kT[:, i * P:(i + 1) * P],
                                 qT[h][:, jq * SL + sc_lo:(jq + 1) * SL],
                                 start=True, stop=True)
                return sc

            for h in range(G):
                o_ps = ps_o.tile([P, SL], f32, tag="o")
                pt_sum = ptsp.tile([P, SL], bf16, tag="ptsum")
                for i in range(ilast + 1):
                    delta, lo_c, _ = spans(i)
                    sp = slice(lo_c, SL)
                    sc = emit_sc(h, i)
                    pt = ptp.tile([P, SL], bf16, tag="pt")
                    nc.scalar.activation(pt[:, sp], sc[:, sp], AF.Exp)
                    if 0 <= delta <= 3:
                        nc.gpsimd.affine_select(
                            out=pt[:, sp], in_=pt[:, sp],
                            compare_op=OP.is_ge, fill=0.0,
                            base=0, pattern=[[1, SL - lo_c]],
                            channel_multiplier=-1)
                    if i == 0:
                        nc.vector.tensor_copy(pt_sum[:], pt[:])
                    else:
                        nc.vector.tensor_add(pt_sum[:, sp], pt_sum[:, sp],
                                             pt[:, sp])
                    nc.tensor.matmul(o_ps[:, sp], v_sb[:, i, :], pt[:, sp],
                                     start=(i == 0), stop=(i == ilast))
                    step += 1
                    if fi < nfill and step % pace == 0:
                        filler[fi]()
                        fi += 1
                # rowsum via single matmul on the DVE-accumulated pt_sum
                rs = ps_nm.tile([P, SL], f32, tag="nm", name=f"rs{jq}_{h}")
                nc.tensor.matmul(rs[:], ones_b[:], pt_sum[:],
                                 start=True, stop=True)
                rb = rbp.tile([P, SL], f32, tag="rb")
                nc.vector.reciprocal(rb[:], rs[:])
                nc.vector.tensor_mul(
                    oT[h][:, jq * SL:(jq + 1) * SL], o_ps[:], rb[:])
            while fi < nfill:
                filler[fi]()
                fi += 1

        # ---- software-pipelined emission ----
        for u in qkv_units(0, with_load=False):
            u()
        load_x(1)
        for u in qkv_units(1, with_load=False):
            u()
        attn(0, qkv_units(2, with_load=True))
        attn(1, qkv_units(3, with_load=True) + [load_wp])
        attn(2, proj_units(0))
        attn(3, proj_units(1) + proj_units(2))
        # final projection slice; last chunk narrow to shorten the tail
        for st in range(SL // P):
            for os_ in range(D // SL):
                if (st, os_) == (SL // P - 1, D // SL - 1):
                    proj_chunk(NSL - 1, st, os_, width=P)
                else:
                    proj_chunk(NSL - 1, st, os_)

    nc.compile()
    return nc


def _rope_tables():
    """cos/sin tables in [HD, S] layout (half-tables stacked twice), plus J^T."""
    inv_freq = 1.0 / (ROPE_BASE ** (np.arange(0, HD, 2, dtype=np.float32) / HD))
    freqs = np.outer(np.arange(S, dtype=np.float32), inv_freq)  # [S, half]
    c = np.cos(freqs).T.astype(np.float32)  # [half, S]
    s = np.sin(freqs).T.astype(np.float32)
    cosf = np.concatenate([c, c], axis=0).copy()  # [HD, S]
    sinf = np.concatenate([s, s], axis=0).copy()
    half = HD // 2
    jT = np.zeros((HD, HD), np.float32)
    jT[np.arange(half) + half, np.arange(half)] = 1.0   # (Jq)[j] = q[j+64], j<64
    jT[np.arange(half), np.arange(half) + half] = -1.0  # (Jq)[j+64] = -q[j]
    return cosf, sinf, jT


def _q8pair(a, scale):
    """Host-side e4m3 two-level quantization of a*scale."""
    import ml_dtypes
    E4 = ml_dtypes.float8_e4m3
    xs = (np.asarray(a, np.float32) * scale).astype(np.float32)
    x8 = xs.astype(E4)
    xr8 = (xs - x8.astype(np.float32)).astype(E4)
    return np.ascontiguousarray(x8), np.ascontiguousarray(xr8)


def make_in_maps(x, Wq, Wk, Wv, Wproj, q_gain):
    """Host-side shard prep: per-core input dicts."""
    import ml_dtypes
    cosT, sinT, jT = _rope_tables()
    xT = np.transpose(np.asarray(x, np.float32), (0, 2, 1))
    x8 = [None] * B
    xr8 = [None] * B
    for b in range(B):
        x8[b], xr8[b] = _q8pair(xT[b], SX)
    Wq = np.asarray(Wq, np.float32)
    Wk = np.asarray(Wk, np.float32)
    Wv = np.asarray(Wv, np.float32)
    # v carries the 2^13 fp8 product scale; fold the inverse into Wproj
    WpT = (np.asarray(Wproj, np.float32).T / SXW).astype(ml_dtypes.bfloat16)
    q_gain = np.asarray(q_gain, np.float32)

    in_maps = []
    for c in range(NCORES):
        b, g = divmod(c, KVH)
        sl_q = slice(g * G * HD, (g + 1) * G * HD)
        sl_kv = slice(g * HD, (g + 1) * HD)
        wq8, wqr8 = _q8pair(Wq[sl_q, :].T, SW)
        wk8, wkr8 = _q8pair(Wk[sl_kv, :].T, SW)
        wv8, wvr8 = _q8pair(Wv[sl_kv, :].T, SW)
        in_maps.append({
            "x8": x8[b],
            "xr8": xr8[b],
            "wq8": wq8, "wqr8": wqr8,
            "wk8": wk8, "wkr8": wkr8,
            "wv8": wv8, "wvr8": wvr8,
            "wpT": np.ascontiguousarray(WpT[sl_q, :]),
            "cosT": cosT,
            "sinT": sinT,
            "jT": jT,
            "qgain": (q_gain[g * G:(g + 1) * G] / np.sqrt(HD))
            .reshape(1, G).astype(np.float32),
        })
    return in_maps


def kernel(x, Wq, Wk, Wv, Wproj, q_gain):
    from concourse.bass_utils import run_bass_kernel_spmd

    if "nc" not in _CACHE:
        _CACHE["nc"] = _build_program()
    nc = _CACHE["nc"]

    in_maps = make_in_maps(x, Wq, Wk, Wv, Wproj, q_gain)
    res = run_bass_kernel_spmd(nc, in_maps, core_ids=list(range(NCORES)))
    _CACHE["last_results"] = res

    y = np.zeros((B, S, D), dtype=np.float32)
    for c in range(NCORES):
        y[c // KVH] += res.results[c]["y"]
    return y
